# revision 23
# baseline (speedup 1.0000x reference)
"""Trainium2 Bass kernel: masked multi-head self-attention block.

out = softmax_mask((x @ Wq) (x @ Wk)^T / sqrt(d)) (x @ Wv) @ Wp + b

Sharding: data-parallel over batch B=8 across the 8 NeuronCores (one
batch row per core); weights replicated; no collectives.

Key compaction: masked keys contribute exactly zero, so each core
keeps only the valid key rows of x (padded to a 128 multiple; padded
slots get a -1e30 score bias -> exp = 0). K/V and attention run on
NK ~= 1152 keys instead of 2048.

v7 structure:
  - All layout work on the host: x pre-transposed/compacted bf16,
    weights bf16, everything pre-shuffled into SBUF-layout [128, ...]
    panels ordered so the prelude's data (wv, wk0, xct) lands first.
  - Every K=128-contraction matmul is emitted as two concurrent K=64
    row-group-packed halves (tile_position via base_partition), so
    LDWEIGHTS overlaps the other half's stream instead of
    serializing.
  - Prelude (PE): V chunks as xct pieces land, K^T[0], Q^T[0] first
    block. Q^T[0] rest, K^T[1..5], Q^T[1..5] run inside the attention
    stream from a backlog using the proj PSUM slot.
  - Attention: flat stream over (qblock, headpair, keychunk) steps,
    S-lookahead 2; exp on ScalarE (j==0 writes the running-sum tile
    directly); running-sum on DVE; col-packed PV; denominator
    matmuls at the next pass's j==1; reciprocal after a DRAM
    broadcast round-trip. The qh=3 passes run hp order
    [5,0,1,2,3,4] and the final proj accumulates c=4 last, so the
    tail only waits on the last head-pair's epilogue.
  - Output stored bf16, upcast to f32 on the host.
"""
import numpy as np
import ml_dtypes

import concourse.bass as bass
import concourse.tile as tile
from concourse import bacc, mybir
from concourse.bass_utils import run_bass_kernel_spmd

F32 = mybir.dt.float32
BF16 = mybir.dt.bfloat16

B, N, DIM = 8, 2048, 768
H, D = 12, 64
SCALE = D ** -0.5
NCH = N // 128        # 16 token chunks
KCH = DIM // 128      # 6 feature chunks
QH = 4                # query blocks
QW = N // QH          # 512 queries per block
Exp = mybir.ActivationFunctionType.Exp
BF16_NP = ml_dtypes.bfloat16


def _kslices(nkc):
    """Key-chunk groups of <=4 (512-key matmul slices)."""
    out = [4] * (nkc // 4)
    if nkc % 4:
        out.append(nkc % 4)
    return out


def _pieces(nkc):
    """Split nkc key chunks into 3 roughly-equal DMA pieces."""
    a = -(-nkc // 3)
    b = -(-(nkc - a) // 2)
    return [a, b, nkc - a - b]


def _build(nc, tc, aps, nkc):
    xT_d, xgT_d, wkv_d, wq_d, wp_d, kb_d, bp_d, o_d = aps
    NK = nkc * 128

    SPLIT_MM = False

    def mm2(out, lhsT, rhs, start, stop, tp_col=None):
        """K=128 contraction as two concurrent K=64 row-group halves."""
        if not SPLIT_MM:
            nc.tensor.matmul(out, lhsT, rhs, start=start, stop=stop)
            return
        for h in range(2):
            nc.tensor.matmul(
                out, lhsT[h * 64:(h + 1) * 64, :],
                rhs[h * 64:(h + 1) * 64, :],
                start=(start and h == 0), stop=(stop and h == 1),
                skip_group_check=True)

    cpool = tc.alloc_tile_pool(name="const", bufs=1)
    gate_t = cpool.tile([1, 8], F32)
    ones_c = cpool.tile([128, 128], BF16)
    nc.vector.memset(ones_c, 0.0)
    nc.vector.memset(ones_c[:, 0:1], 1.0)
    kb_t = cpool.tile([128, nkc], F32)
    bp_bc = cpool.tile([128, DIM], F32)

    # persistent tiles (live through attention; released at the end)
    qkvpool = tc.alloc_tile_pool(name="qkv_sb", bufs=1)
    qt = [qkvpool.tile([128, N], BF16, tag=f"qt{m}", name=f"qt{m}")
          for m in range(KCH)]
    kt = [qkvpool.tile([128, NK], BF16, tag=f"kt{m}", name=f"kt{m}")
          for m in range(KCH)]
    v_nat = [qkvpool.tile([128, DIM], BF16, tag=f"vn{t}", name=f"vn{t}")
             for t in range(nkc)]
    ot = [[qkvpool.tile([128, QW], BF16, tag=f"ot{c}_{q}", name=f"ot{c}_{q}")
           for q in range(QH)] for c in range(KCH)]
    # big packed panels (host-shuffled layouts)
    xct_all = qkvpool.tile([128, nkc * DIM], BF16, tag="xct", name="xct")
    wkv_sb = qkvpool.tile([128, KCH * 1536], BF16, tag="wkv", name="wkv")
    xt_all = qkvpool.tile([128, KCH * N], BF16, tag="xt", name="xt")
    wq_sb = qkvpool.tile([128, KCH * DIM], BF16, tag="wqq", name="wqq")
    wp_sb = qkvpool.tile([128, KCH * DIM], BF16, tag="wpp", name="wpp")

    # xct: key-chunk-major [128, (t c k)]
    xck = xct_all.rearrange("p (t c k) -> p t c k", t=nkc, c=KCH)

    def xtb(c, blk):
        o = blk * (KCH * QW) + c * QW
        return xt_all[:, o:o + QW]

    def wv(c, lo, hi):
        o = c * 768
        return wkv_sb[:, o + lo:o + hi]

    def wk(c, m):
        if m == 0:
            o = KCH * 768 + c * 128
        else:
            o = KCH * 768 + KCH * 128 + (m - 1) * 768 + c * 128
        return wkv_sb[:, o:o + 128]

    def wqc(c, m):
        o = m * 768 + c * 128
        return wq_sb[:, o:o + 128]

    def wp(c, lo, hi):
        o = c * DIM
        return wp_sb[:, o + lo:o + hi]

    # DMA issue order (FIFO per ring; rings round-robin): the prelude's
    # inputs (kb, wv+wk0, xct pieces, wq m=0) go first on each ring.
    nc.sync.dma_start(out=kb_t, in_=kb_d)
    wvk0 = KCH * 768 + KCH * 128
    nc.sync.dma_start(out=wkv_sb[:, 0:wvk0], in_=wkv_d[:, 0:wvk0])
    pc = _pieces(nkc)
    p0w = pc[0] * DIM
    nc.gpsimd.dma_start(out=xct_all[:, 0:p0w], in_=xgT_d[:, 0:p0w])
    # gate: issue the rest of xct only once piece 0 has landed, and the
    # Q0 inputs only once wv has landed, so the first V chunk's inputs
    # get the full DMA bandwidth
    nc.gpsimd.tensor_copy(gate_t[0:1, 0:1], xct_all[0:1, p0w - 1:p0w])
    off = p0w
    for p in pc[1:]:
        w = p * DIM
        nc.gpsimd.dma_start(out=xct_all[:, off:off + w],
                            in_=xgT_d[:, off:off + w])
        off += w
    nc.scalar.copy(gate_t[0:1, 1:2], wkv_sb[0:1, wvk0 - 1:wvk0])
    nc.scalar.dma_start(out=wq_sb[:, 0:768], in_=wq_d[:, 0:768])
    nc.scalar.dma_start(out=xt_all[:, 0:KCH * QW], in_=xT_d[:, 0:KCH * QW])

    # ---- prelude compute: V chunks (piecewise), K^T[0], Q^T[0] blk 0
    def v_chunk(ps_v, t):
        v_ps = ps_v.tile([128, 2, 512], F32, tag="v_ps", name="v_ps")
        for c in range(KCH):
            mm2(v_ps[:, 0, :], xck[:, t, c, :], wv(c, 0, 512),
                start=(c == 0), stop=(c == KCH - 1))
            mm2(v_ps[:, 1, 0:256], xck[:, t, c, :], wv(c, 512, 768),
                start=(c == 0), stop=(c == KCH - 1))
        nc.vector.tensor_copy(v_nat[t][:, 0:512], v_ps[:, 0, :])
        nc.vector.tensor_copy(v_nat[t][:, 512:DIM], v_ps[:, 1, 0:256])

    with tc.tile_pool(name="ps_v", bufs=2, space="PSUM") as ps_v, \
         tc.tile_pool(name="ps_k", bufs=1, space="PSUM") as ps_k:
        for t in range(pc[0]):
            v_chunk(ps_v, t)
        # K^T[0]
        mm_ps = ps_k.tile([128, NK], F32, tag="k_ps", name="k_ps")
        t0 = 0
        for nt in _kslices(nkc):
            lo = t0 * 128
            for c in range(KCH):
                mm2(mm_ps[:, lo:lo + nt * 128], wk(c, 0),
                    xck[:, t0:t0 + nt, c, :],
                    start=(c == 0), stop=(c == KCH - 1))
            t0 += nt
        nc.scalar.copy(kt[0], mm_ps)
        # bulk loads issue only now (scalar queue was busy until the
        # kt[0] copy), leaving the full DMA bandwidth to the prelude
        nc.scalar.dma_start(out=wkv_sb[:, wvk0:], in_=wkv_d[:, wvk0:])
        nc.scalar.dma_start(out=wq_sb[:, 768:], in_=wq_d[:, 768:])
        nc.scalar.dma_start(out=xt_all[:, KCH * QW:], in_=xT_d[:, KCH * QW:])
        nc.scalar.dma_start(out=wp_sb, in_=wp_d)
        nc.scalar.dma_start(out=bp_bc, in_=bp_d)
        for t in range(pc[0], nkc):
            v_chunk(ps_v, t)
    with tc.tile_pool(name="ps_q0", bufs=1, space="PSUM") as ps_q0:
        mm_ps = ps_q0.tile([128, QW], F32, tag="q_ps", name="q_ps")
        for c in range(KCH):
            mm2(mm_ps, wqc(c, 0), xtb(c, 0),
                start=(c == 0), stop=(c == KCH - 1))
        nc.scalar.copy(qt[0][:, 0:QW], mm_ps)

    # deferred Q^T[0] rest + K^T/Q^T chunks (inside attention stream,
    # borrowing the proj PSUM slot)
    def qk_backlog(ps):
        items = []
        for blk in range(1, QH):
            st = {}

            def q0step(cc, st=st, blk=blk, first=False):
                if first:
                    st["ps"] = ps.tile([128, QW], F32, tag="pr",
                                       bufs=1, name="kq_ps")
                for c in cc:
                    mm2(st["ps"], wqc(c, 0), xtb(c, blk),
                        start=(c == 0), stop=(c == KCH - 1))

            def fin_q0(st=st, blk=blk):
                nc.vector.tensor_copy(qt[0][:, blk * QW:(blk + 1) * QW],
                                      st["ps"])

            items.append(lambda f=q0step: f((0, 1), first=True))
            items.append(lambda f=q0step: f((2, 3)))
            items.append(lambda f=q0step: f((4, 5)))
            items.append(fin_q0)
        ksl = _kslices(nkc)
        for m in range(1, KCH):
            t0 = 0
            for nt in ksl:
                st = {}

                def kstep(cc, st=st, m=m, t0=t0, nt=nt, first=False):
                    if first:
                        st["ps"] = ps.tile([128, nt * 128], F32, tag="pr",
                                           bufs=1, name="kq_ps")
                    for c in cc:
                        mm2(st["ps"], wk(c, m), xck[:, t0:t0 + nt, c, :],
                            start=(c == 0), stop=(c == KCH - 1))

                def fin_k(st=st, m=m, t0=t0, nt=nt):
                    nc.vector.tensor_copy(
                        kt[m][:, t0 * 128:(t0 + nt) * 128], st["ps"])

                items.append(lambda f=kstep: f((0, 1), first=True))
                items.append(lambda f=kstep: f((2, 3)))
                items.append(lambda f=kstep: f((4, 5)))
                items.append(fin_k)
                t0 += nt
            for half in range(2):
                st = {}
                lo = half * 1024

                def qstep(c, st=st, m=m, lo=lo, first=False):
                    if first:
                        st["ps"] = ps.tile([128, 1024], F32, tag="pr",
                                           bufs=1, name="kq_ps")
                    for g in range(2):
                        mm2(st["ps"][:, g * 512:(g + 1) * 512],
                            wqc(c, m), xtb(c, lo // 512 + g),
                            start=(c == 0), stop=(c == KCH - 1))

                def fin_q(st=st, m=m, lo=lo):
                    nc.vector.tensor_copy(qt[m][:, lo:lo + 1024], st["ps"])

                for c in range(KCH):
                    items.append(lambda c=c, f=qstep: f(c, first=(c == 0)))
                items.append(fin_q)
        return items

    _attention(nc, tc, mm2, qt, kt, v_nat, kb_t, ones_c, ot, nkc, wp,
               bp_bc, o_d, qk_backlog)
    qkvpool.release()
    cpool.release()


SPLIT_PV = False


def _attention(nc, tc, mm2, qt, kt, v_nat, kb_t, ones_c, ot, nkc,
               wp, bp_bc, o_d, qk_backlog):
    with tc.tile_pool(name="p_sb", bufs=4) as ppool, \
         tc.tile_pool(name="rs_sb", bufs=2) as rspool, \
         tc.tile_pool(name="ep_sb", bufs=3) as eppool, \
         tc.tile_pool(name="out_sb", bufs=3) as outpool, \
         tc.tile_pool(name="dr_sb", bufs=3, space="DRAM") as drpool, \
         tc.tile_pool(name="ps_c", bufs=1, space="PSUM") as ps:

        backlog = qk_backlog(ps)
        drained = [0]

        def drain(k):
            for _ in range(min(k, len(backlog))):
                backlog.pop(0)()
                drained[0] += 1

        def emit_S(qh, hp, j):
            q0 = qh * QW
            s_t = ps.tile([128, 2, 512], F32, tag="s", bufs=2, name="s_t")
            for a in range(2):
                r0 = a * 64
                nc.tensor.matmul(
                    s_t[:, a, :],
                    kt[hp][r0:r0 + 64, j * 128:(j + 1) * 128],
                    qt[hp][r0:r0 + 64, q0:q0 + QW],
                    start=True, stop=True)
            return s_t

        def queue_proj(qh):
            corder = list(range(KCH))
            if qh == QH - 1:
                # queued at pass (3,3); c=3 ready at (3,4) j==3, c=4 last
                corder = [0, 1, 2, 5, 3, 4]

            def make_chunk(t_i, tag):
                st = {}

                def cstep(ci, t_i=t_i, st=st, tag=tag):
                    c = corder[ci]
                    if ci == 0:
                        if tag == "s":
                            st["pr"] = ps.tile([128, 2, 512], F32, tag="s",
                                               bufs=2, name="pr")
                        else:
                            st["pr"] = ps.tile([128, 2, 512], F32,
                                               tag="pr", bufs=1, name="pr")
                    tl = (t_i % 4) * 128
                    pr = st["pr"]
                    mm2(pr[:, 0, :], ot[c][t_i // 4][:, tl:tl + 128],
                        wp(c, 0, 512),
                        start=(ci == 0), stop=(ci == KCH - 1))
                    mm2(pr[:, 1, 0:256], ot[c][t_i // 4][:, tl:tl + 128],
                        wp(c, 512, DIM),
                        start=(ci == 0), stop=(ci == KCH - 1))

                def finish(t_i=t_i, st=st):
                    pr = st["pr"]
                    out_t = outpool.tile([128, DIM], BF16, tag="out_t",
                                         name="out_t")
                    nc.vector.tensor_add(out_t[:, 0:512], pr[:, 0, :],
                                         bp_bc[:, 0:512])
                    nc.vector.tensor_add(out_t[:, 512:DIM], pr[:, 1, 0:256],
                                         bp_bc[:, 512:DIM])
                    nc.sync.dma_start(
                        out=o_d[t_i * 128:(t_i + 1) * 128, :], in_=out_t)

                return cstep, finish

            if qh < QH - 1:
                for ti in range(4):
                    cstep, finish = make_chunk(qh * 4 + ti, "pr")
                    for ci in range(KCH):
                        backlog.append(lambda ci=ci, f=cstep: f(ci))
                    backlog.append(finish)
            else:
                # c-major across 3 concurrent chunks, then the 4th; the
                # corder puts c=4 (last epilogue's head-pair) last
                chunks = [make_chunk(qh * 4 + ti,
                                     ("s", "s", "pr")[ti])
                          for ti in range(3)]
                for ci in range(KCH):
                    for cstep, _ in chunks:
                        backlog.append(lambda ci=ci, f=cstep: f(ci))
                for _, finish in chunks:
                    backlog.append(finish)
                cstep, finish = make_chunk(qh * 4 + 3, "s")
                for ci in range(KCH):
                    backlog.append(lambda ci=ci, f=cstep: f(ci))
                backlog.append(finish)

        pending_dn = []
        pending_ep = []

        def epilogue(qh, hp, rs_t, o_t):
            def part1(qh=qh, hp=hp, rs_t=rs_t, o_t=o_t):
                dn_t = ps.tile([128, 2, 512], F32, tag="s", bufs=2,
                               name="dn_t")
                for a in range(2):
                    mm2(dn_t[:, a, :], ones_c, rs_t[:, a, :],
                        start=True, stop=True)
                dn_sb = eppool.tile([1, 2, 512], F32, tag="dn_sb",
                                    name="dn_sb")
                nc.vector.tensor_copy(dn_sb, dn_t[0:1, :, :])
                rc_dram = drpool.tile([1024], F32, tag="rc_dram",
                                      name="rc_dram")
                nc.sync.dma_start(out=rc_dram, in_=dn_sb)
                b_raw = eppool.tile([128, QW], F32, tag="b_raw",
                                    name="b_raw", bufs=3)
                for a in range(2):
                    bc_ap = bass.AP(
                        tensor=rc_dram.tensor,
                        offset=rc_dram.offset + a * 512,
                        ap=[[0, 64], [1, 512]])
                    nc.sync.dma_start(out=b_raw[a * 64:(a + 1) * 64, :],
                                      in_=bc_ap)

                def part2(qh=qh, hp=hp, o_t=o_t, b_raw=b_raw):
                    rc_b = eppool.tile([128, QW], F32, tag="rc_b",
                                       name="rc_b", bufs=3)
                    nc.vector.reciprocal_approx_fast(out=rc_b, in_=b_raw)
                    nc.vector.tensor_mul(ot[hp][qh], o_t, rc_b)

                pending_ep.append(part2)

            pending_dn.append(part1)

        tri = [(qh, hp) for hp in range(H // 2) for qh in (0, 1, 2)]
        rest = [(3, 5)] + [(3, hp) for hp in range(H // 2 - 1)]
        steps = [(qh, hp, j) for qh, hp in tri + rest
                 for j in range(nkc)]
        # minimum backlog items that must be drained before the first S
        # of each pass is emitted: Q0 block qh (4 items each) and the
        # K_m/Q_m chains (4*nsl + 14 items per m) up to m = hp.
        nsl = len(_kslices(nkc))
        per_m = 4 * nsl + 2 * (KCH + 1)
        req_at = {}
        for p, (pqh, php) in enumerate(tri + rest):
            r = max(4 * pqh, (12 + per_m * php) if php >= 1 else 0)
            req_at[p * nkc] = r
        s_pend = {}
        s_pend[0] = emit_S(*steps[0])
        s_pend[1] = emit_S(*steps[1])
        hp_state = {}
        for idx, (qh, hp, j) in enumerate(steps):
            if j == 0:
                o_t = ps.tile([128, QW], F32, tag="o", bufs=2, name="o_t")
                rs_t = rspool.tile([128, 2, 512], BF16, tag="rs",
                                   name="rs_t")
                hp_state[(qh, hp)] = (o_t, rs_t)
            o_t, rs_t = hp_state[(qh, hp)]
            # spread drains so requirement points are met without bursts
            tgt = 0
            for e, r in req_at.items():
                if e > idx + 2:
                    tgt = max(tgt, r - (e - idx - 3))
            need = req_at.get(idx + 2)
            if need is not None:
                tgt = max(tgt, need)
            if drained[0] < tgt:
                drain(tgt - drained[0])
            s_t = s_pend.pop(idx)
            if j == 0:
                # first chunk's exp writes the running-sum tile directly
                pt_t = rs_t
            else:
                pt_t = ppool.tile([128, 2, 512], BF16, tag="pt",
                                  name="pt_t")
            nc.scalar.activation(pt_t, s_t, Exp,
                                 bias=kb_t[:, j:j + 1], scale=SCALE)
            if idx + 2 < len(steps):
                s_pend[idx + 2] = emit_S(*steps[idx + 2])
            if j > 0:
                nc.vector.tensor_add(rs_t, rs_t, pt_t)
            for a in range(2):
                h = 2 * hp + a
                if SPLIT_PV:
                    for kh in range(2):
                        nc.tensor.matmul(
                            o_t[a * 64:(a + 1) * 64, :],
                            v_nat[j][kh * 64:(kh + 1) * 64,
                                     h * D:(h + 1) * D],
                            pt_t[kh * 64:(kh + 1) * 64, a, :],
                            start=(j == 0 and kh == 0),
                            stop=(j == nkc - 1 and kh == 1),
                            tile_position=(kh * 64, a * 64),
                            skip_group_check=True)
                else:
                    nc.tensor.matmul(
                        o_t[a * 64:(a + 1) * 64, :],
                        v_nat[j][:, h * D:(h + 1) * D],
                        pt_t[:, a, :],
                        start=(j == 0), stop=(j == nkc - 1),
                        tile_position=(0, a * 64),
                        skip_group_check=True)
            if j == 1:
                while pending_dn:
                    pending_dn.pop(0)()
            if j == 3:
                while pending_ep:
                    pending_ep.pop(0)()
            if j == nkc - 1:
                epilogue(qh, hp, rs_t, o_t)
                del hp_state[(qh, hp)]
                if (qh < QH - 1 and hp == H // 2 - 1) or \
                        (qh == QH - 1 and hp == 4):
                    queue_proj(qh)
            if j in (1, 3, 4):
                pass  # keep PE light where exp stalls cluster
            elif j in (6, 7, 8) and len(backlog) > len(steps) - idx:
                drain(2)
            else:
                drain(1)
        while pending_dn:
            pending_dn.pop(0)()
        while pending_ep:
            pending_ep.pop(0)()
        drain(len(backlog))


_CACHE = {}


def _get_compiled(nkc):
    if nkc in _CACHE:
        return _CACHE[nkc]
    NK = nkc * 128
    nc = bacc.Bacc("TRN2", target_bir_lowering=False, debug=False,
                   num_devices=B)
    xT_d = nc.dram_tensor("xT", [128, KCH * N], BF16,
                          kind="ExternalInput").ap()
    xgT_d = nc.dram_tensor("xgT", [128, nkc * DIM], BF16,
                           kind="ExternalInput").ap()
    wkv_d = nc.dram_tensor("w_kv", [128, KCH * 1536], BF16,
                           kind="ExternalInput").ap()
    wq_d = nc.dram_tensor("w_q", [128, KCH * DIM], BF16,
                          kind="ExternalInput").ap()
    wp_d = nc.dram_tensor("w_p", [128, KCH * DIM], BF16,
                          kind="ExternalInput").ap()
    kb_d = nc.dram_tensor("kb", [128, nkc], F32,
                          kind="ExternalInput").ap()
    bp_d = nc.dram_tensor("bp", [128, DIM], F32,
                          kind="ExternalInput").ap()
    o_d = nc.dram_tensor("out", [N, DIM], BF16, kind="ExternalOutput").ap()
    with tile.TileContext(nc) as tc:
        _build(nc, tc, (xT_d, xgT_d, wkv_d, wq_d, wp_d, kb_d, bp_d, o_d),
               nkc)
    nc.compile()
    _CACHE[nkc] = nc
    return nc


def _pack_panel(a):
    """[KCH*128, W] -> [128, KCH*W]: row-chunk c lands at cols c*W."""
    w = a.shape[1]
    return np.ascontiguousarray(
        a.reshape(KCH, 128, w).transpose(1, 0, 2).reshape(128, KCH * w))


def prep_run(x, mask, w_qkv, w_proj, b_proj):
    """Build the compiled program + per-core input maps."""
    x = np.asarray(x, dtype=np.float32)
    mask = np.asarray(mask, dtype=np.int32)

    idxs = [np.flatnonzero(mask[b]).astype(np.int32) for b in range(B)]
    max_valid = max(len(i) for i in idxs)
    nkc = min(NCH, max(1, -(-max_valid // 128)))
    NK = nkc * 128

    xbf = x.astype(BF16_NP)
    wq_np = np.asarray(w_qkv, dtype=np.float32).astype(BF16_NP)
    # wkv panel: [wv (c-major) | wk m=0 (c-major) | wk m=1..5 (m,c-major)]
    wv_p = _pack_panel(wq_np[:, 2 * DIM:3 * DIM])            # [128, 4608]
    wk_r = wq_np[:, DIM:2 * DIM].reshape(KCH, 128, KCH, 128)
    wk_mc = wk_r.transpose(1, 2, 0, 3).reshape(128, KCH * DIM)  # m,c-major
    wkv_p = np.ascontiguousarray(
        np.concatenate([wv_p, wk_mc], axis=1))               # [128, 9216]
    wqq_r = wq_np[:, 0:DIM].reshape(KCH, 128, KCH, 128)
    wq_p = np.ascontiguousarray(
        wqq_r.transpose(1, 2, 0, 3).reshape(128, KCH * DIM))  # m,c-major
    wp_p = _pack_panel(np.asarray(w_proj, dtype=np.float32).astype(BF16_NP))
    bp_t = np.ascontiguousarray(
        np.tile(np.asarray(b_proj, dtype=np.float32)[None, :], (128, 1)))

    in_maps = []
    for b in range(B):
        n = len(idxs[b])
        kidx = np.zeros(NK, dtype=np.int32)
        kidx[:n] = idxs[b]
        kbias = np.full(NK, -1.0e30, dtype=np.float32)
        kbias[:n] = 0.0
        # xT: query-block-major [128, (blk c tok)]
        xTb = xbf[b].T.reshape(KCH, 128, QH, QW)
        xT = np.ascontiguousarray(
            xTb.transpose(1, 2, 0, 3).reshape(128, KCH * N))
        # xgT: key-chunk-major [128, (t c k)]
        xg = xbf[b][kidx].T.reshape(KCH, 128, nkc, 128)
        xgT = np.ascontiguousarray(
            xg.transpose(1, 2, 0, 3).reshape(128, nkc * DIM))
        kb = np.ascontiguousarray(kbias.reshape(nkc, 128).T)
        in_maps.append({"xT": xT, "xgT": xgT, "w_kv": wkv_p,
                        "w_q": wq_p, "w_p": wp_p, "kb": kb, "bp": bp_t})

    nc = _get_compiled(nkc)
    return nc, in_maps


def kernel(x, mask, w_qkv, w_proj, b_proj):
    nc, in_maps = prep_run(x, mask, w_qkv, w_proj, b_proj)
    last_err = None
    for _ in range(3):
        try:
            res = run_bass_kernel_spmd(nc, in_maps, list(range(B))).results
            return np.stack(
                [res[b]["out"].astype(np.float32) for b in range(B)],
                axis=0)
        except Exception as e:  # transient device hiccup: retry
            last_err = e
    raise last_err
